# revision 19
# baseline (speedup 1.0000x reference)
"""Trainium2 Bass kernel for nn_Decoder (additive-attention LSTM decoder).

Data-parallel over batch: 1024 rows split as 128 per NeuronCore across 8 cores.
All on-chip layouts keep feature dims on partitions and batch on the free dim,
so the LSTM state never needs an on-chip transpose.

v2 restructure vs baseline:
- per-tq (16-step t-quarter) score psum groups + streaming exp (no reduce_max:
  |scores| <= sum|W_a3| ~ 20 so raw exp is f32-safe) -> softmax tail collapses
- merged 8192-wide tanh per tq for tq1..3 (less ACT instr overhead); tq0 keeps
  per-chunk tanhs so the first tanh starts right after one z1 e-chunk
- z1 matmuls grouped per e-chunk (psum start/stop per m) with per-chunk psum
  evacuation, c-part emitted before h-part so it can overlap the LSTM tail
- W_hh matmuls emitted mid-phase (PE is idle between score bursts)
- LSTM gate order [i,f,g,o] with g-weights pre-doubled so one ACT op computes
  tanh(0.5*[i,f,2g]) = [sig-halves for i,f and tanh(g)] in one go
- step 0 skips z1/whh matmuls entirely (h=c=0)
"""

import os
import numpy as np

B, T, E, D = 1024, 64, 512, 512
NCORES = 8
BL = B // NCORES          # 128 batch rows per core
EC = E // 128             # 4 e-chunks
KD = (2 * D) // 128       # 8 contraction chunks for z1
GB = (4 * D) // 128       # 16 gate blocks
TQ = 4                    # t-quarters
TTQ = T // TQ             # 16 t per quarter
N_STEPS = int(os.environ.get("KERNEL_N_STEPS", str(T)))

_PROG_CACHE = {}


def _build_program(n_steps, wfcy, bfc, bff):
    from contextlib import ExitStack

    import concourse.bass as bass
    import concourse.tile as tile
    from concourse import bacc, mybir

    f16 = mybir.dt.float16
    f32 = mybir.dt.float32
    AF = mybir.ActivationFunctionType
    OP = mybir.AluOpType
    AX = mybir.AxisListType

    nc = bacc.Bacc("TRN2", target_bir_lowering=False, debug=False)

    xt_d = nc.dram_tensor("xt", (128, EC * T * 128), f16, kind="ExternalInput")
    y_d = nc.dram_tensor("yh", (BL, T), f32, kind="ExternalInput")
    wa1_d = nc.dram_tensor("wa1t", (128, KD * 512), f16, kind="ExternalInput")
    wa2_d = nc.dram_tensor("wa2t", (128, EC * 512), f16, kind="ExternalInput")
    wa3_d = nc.dram_tensor("wa3", (128, EC), f16, kind="ExternalInput")
    whh_d = nc.dram_tensor("whht", (128, 4 * 2048), f16, kind="ExternalInput")
    wihb_d = nc.dram_tensor("wihb", (128, 2048), f16, kind="ExternalInput")
    bias1_d = nc.dram_tensor("bias1", (128, EC), f32, kind="ExternalInput")
    wfc2_d = nc.dram_tensor("wfc2", (128, 2 * EC), f16, kind="ExternalInput")
    wffh_d = nc.dram_tensor("wffh", (128, EC), f16, kind="ExternalInput")
    ident_d = nc.dram_tensor("ident", (128, 128), f32, kind="ExternalInput")
    ytinit_d = nc.dram_tensor("ytinit", (128, 128), f16, kind="ExternalInput")
    out_d = nc.dram_tensor("out", (BL, 1), f32, kind="ExternalOutput")

    with tile.TileContext(nc) as tc, ExitStack() as ctx:
        const = ctx.enter_context(tc.tile_pool(name="const", bufs=1))
        z2pool = ctx.enter_context(tc.tile_pool(name="z2pool", bufs=1))

        # ---- constants into SBUF ----
        wa1t = const.tile([128, KD * 512], f16, name="wa1t", tag="wa1t")
        nc.sync.dma_start(wa1t[:], wa1_d.ap())
        whht = const.tile([128, 4 * 2048], f16, name="whht", tag="whht")
        nc.sync.dma_start(whht[:], whh_d.ap())
        wa3s = const.tile([128, EC], f16, name="wa3s", tag="wa3s")
        nc.sync.dma_start(wa3s[:], wa3_d.ap())
        wihb = const.tile([128, 2048], f16, name="wihb", tag="wihb")
        nc.sync.dma_start(wihb[:], wihb_d.ap())
        bias1 = const.tile([128, EC], f32, name="bias1", tag="bias1")
        nc.sync.dma_start(bias1[:], bias1_d.ap())
        wffh = const.tile([128, EC], f16, name="wffh", tag="wffh")
        nc.sync.dma_start(wffh[:], wffh_d.ap())
        ident = const.tile([128, 128], f32, name="ident", tag="ident")
        nc.sync.dma_start(ident[:], ident_d.ap())
        ysb = const.tile([BL, T], f32, name="ysb", tag="ysb")
        nc.sync.dma_start(ysb[:], y_d.ap())

        ytw = const.tile([BL, T], f32, name="ytw", tag="ytw")
        nc.vector.tensor_scalar(ytw[:], ysb[:], float(wfcy), float(bfc),
                                OP.mult, OP.add)

        xw = const.tile([BL, T], f32, name="xw", tag="xw")
        xw2 = const.tile([BL, T], f32, name="xw2", tag="xw2")
        zf16 = const.tile([128, 512], f16, name="zf16", tag="zf16")
        nc.vector.memset(zf16[:], 0.0)

        # z2 in transposed layout: z2all[p, c*8192 + t*128 + b]
        z2all = z2pool.tile([128, EC * T * 128], f16, name="z2all", tag="z2all")

        # ---- precompute phase: z2 = x @ W_a2.T, xw = x.W_fc, xw2 = x.W_ff2 ----
        with tc.tile_pool(name="xtp", bufs=1) as xtp, \
             tc.tile_pool(name="pcps", bufs=4, space="PSUM") as pcps:
            xts = xtp.tile([128, EC * T * 128], f16, name="xts", tag="xts")
            nc.sync.dma_start(xts[:], xt_d.ap())
            wa2t = xtp.tile([128, EC * 512], f16, name="wa2t", tag="wa2t")
            nc.sync.dma_start(wa2t[:], wa2_d.ap())
            wfc2 = xtp.tile([128, 2 * EC], f16, name="wfc2", tag="wfc2")
            nc.sync.dma_start(wfc2[:], wfc2_d.ap())

            # z2
            for cf in range(EC):
                for n in range(16):
                    zp = pcps.tile([128, 512], f32, name="zp", tag="zp")
                    for k in range(EC):
                        nc.tensor.matmul(
                            zp[:],
                            wa2t[:, k * 512 + cf * 128:k * 512 + (cf + 1) * 128],
                            xts[:, k * 8192 + n * 512:k * 8192 + (n + 1) * 512],
                            start=(k == 0), stop=(k == EC - 1))
                    nc.vector.tensor_copy(
                        z2all[:, cf * 8192 + n * 512:cf * 8192 + (n + 1) * 512],
                        zp[:])

            # xw / xw2: out[b, 2t:2t+2] = sum_e xT[e, t, b] * wfc2[e, :]
            xwp = pcps.tile([128, 2 * T], f32, name="xwp", tag="xwp", bufs=1)
            for t in range(T):
                for k in range(EC):
                    nc.tensor.matmul(
                        xwp[:, 2 * t:2 * t + 2],
                        xts[:, k * 8192 + t * 128:k * 8192 + (t + 1) * 128],
                        wfc2[:, 2 * k:2 * k + 2],
                        start=(k == 0 and t == 0),
                        stop=(k == EC - 1 and t == T - 1))
            xwp3 = xwp.rearrange("p (t two) -> p t two", two=2)
            nc.vector.tensor_copy(xw[:], xwp3[:, :, 0])
            nc.vector.tensor_copy(xw2[:], xwp3[:, :, 1])

        # Loop-phase pools open after the precompute pools released their space.
        state = ctx.enter_context(tc.tile_pool(name="state", bufs=1))
        qbig = ctx.enter_context(tc.tile_pool(name="qbig", bufs=3))
        qsm = ctx.enter_context(tc.tile_pool(name="qsm", bufs=6))
        work = ctx.enter_context(tc.tile_pool(name="work", bufs=2))
        gpsum = ctx.enter_context(
            tc.tile_pool(name="gpsum", bufs=1, space="PSUM"))
        ps1 = ctx.enter_context(tc.tile_pool(name="ps1", bufs=1, space="PSUM"))

        # ---- LSTM state (packed transposed layout, doubled h and c) ----
        hT = state.tile([128, 512], f16, name="hT", tag="hT")
        nc.vector.memset(hT[:], 0.0)
        cD = state.tile([128, 512], f32, name="cD", tag="cD")
        nc.vector.memset(cD[:], 0.0)
        cT16 = state.tile([128, 512], f16, name="cT16", tag="cT16")
        nc.vector.memset(cT16[:], 0.0)
        ytones = state.tile([128, 128], f16, name="ytones", tag="ytones")
        nc.sync.dma_start(ytones[:], ytinit_d.ap())

        e4 = None
        rden = None

        for s in range(n_steps):
            # ---- z1_T packed psum, per-m accumulation groups ----
            z1p = work.tile([128, 512], f16, name="z1p", tag="z1p")
            if s == 0:
                # h = c = 0: z1 = bias only
                nc.vector.tensor_tensor(
                    z1p.rearrange("p (m b) -> p m b", m=EC),
                    zf16.rearrange("p (m b) -> p m b", m=EC),
                    bias1.unsqueeze(2).broadcast_to((128, EC, 128)),
                    op=OP.add)
            else:
                # two independent psum half-tiles so the evac of one half
                # never blocks the other half's matmuls (coarse WAR)
                z1h = [ps1.tile([128, 256], f32, name="z1a", tag="z1a"),
                       ps1.tile([128, 256], f32, name="z1b", tag="z1b")]
                for half in range(2):
                    zt = z1h[half]
                    for mm_ in range(2):
                        m = half * 2 + mm_
                        # c-part first (cT16 ready before hT in prev tail)
                        for k in range(4, KD):
                            nc.tensor.matmul(
                                zt[:, mm_ * 128:(mm_ + 1) * 128],
                                wa1t[:, k * 512 + m * 128:k * 512 + (m + 1) * 128],
                                cT16[:, (k - 4) * 128:(k - 3) * 128],
                                start=(k == 4), stop=False)
                        for k in range(4):
                            nc.tensor.matmul(
                                zt[:, mm_ * 128:(mm_ + 1) * 128],
                                wa1t[:, k * 512 + m * 128:k * 512 + (m + 1) * 128],
                                hT[:, k * 128:(k + 1) * 128],
                                start=False, stop=(k == 3))
                    # evacuate half (2 e-chunks) with bias fold -> f16 sbuf
                    nc.vector.tensor_tensor(
                        z1p[:, half * 256:(half + 1) * 256]
                            .rearrange("p (m b) -> p m b", m=2),
                        zt.rearrange("p (m b) -> p m b", m=2),
                        bias1[:, half * 2:half * 2 + 2].unsqueeze(2)
                            .broadcast_to((128, 2, 128)),
                        op=OP.add)

            gps = gpsum.tile([128, 2048], f32, name="gps", tag="gps")
            scps = ps1.tile([128, T], f32, name="scps", tag="scps")
            e4 = work.tile([BL, T], f32, name="e4", tag="e4")
            den4 = work.tile([BL, TQ], f32, name="den4", tag="den4")
            tmp64 = work.tile([BL, T], f32, name="tmp64", tag="tmp64")

            def emit_add(qt_ap, tq, c):
                base = c * 8192 + tq * TTQ * 128
                nc.vector.tensor_tensor(
                    qt_ap.rearrange("p (t b) -> p t b", t=TTQ),
                    z2all[:, base:base + TTQ * 128]
                        .rearrange("p (t b) -> p t b", t=TTQ),
                    z1p[:, c * 128:(c + 1) * 128].unsqueeze(1)
                        .broadcast_to((128, TTQ, 128)),
                    op=OP.add)

            for tq in range(TQ):
                if tq in (0, 3):
                    # per-chunk small tiles: tq0 so the first tanh starts
                    # after one add; tq3 with per-c score rounds so the
                    # step tail is just 16 matmuls + exp
                    qs = []
                    for c in range(EC):
                        qt = qsm.tile([128, TTQ * 128], f16, name="qs",
                                      tag="qs")
                        emit_add(qt[:], tq, c)
                        nc.scalar.activation(qt[:], qt[:], AF.Tanh)
                        qs.append(qt)
                    for tt in range(TTQ):
                        t_g = tq * TTQ + tt
                        for c in range(EC):
                            nc.tensor.matmul(
                                scps[:, t_g:t_g + 1],
                                qs[c][:, tt * 128:(tt + 1) * 128],
                                wa3s[:, c:c + 1],
                                start=(c == 0), stop=(c == EC - 1))
                else:
                    qt = qbig.tile([128, TQ * TTQ * 128], f16, name="qb",
                                   tag="qb")
                    for c in range(EC):
                        emit_add(qt[:, c * 2048:(c + 1) * 2048], tq, c)
                    nc.scalar.activation(qt[:], qt[:], AF.Tanh)
                    for tt in range(TTQ):
                        t_g = tq * TTQ + tt
                        for c in range(EC):
                            nc.tensor.matmul(
                                scps[:, t_g:t_g + 1],
                                qt[:, c * 2048 + tt * 128:c * 2048 + (tt + 1) * 128],
                                wa3s[:, c:c + 1],
                                start=(c == 0), stop=(c == EC - 1))

                if tq == 0 and s > 0:
                    # W_hh gate matmuls: PE is idle between score bursts
                    for m in range(GB):
                        for k in range(4):
                            nc.tensor.matmul(
                                gps[:, m * 128:(m + 1) * 128],
                                whht[:, k * 2048 + m * 128:k * 2048 + (m + 1) * 128],
                                hT[:, k * 128:(k + 1) * 128],
                                start=(k == 0 and m % 4 == 0), stop=False)

                # streaming exp (no max subtraction needed: |s| <= sum|W_a3|)
                tqr = slice(tq * TTQ, (tq + 1) * TTQ)
                nc.scalar.activation(e4[:, tqr], scps[:, tqr], AF.Exp,
                                     accum_out=den4[:, tq:tq + 1])

            # ---- y_tilde ----
            ynum = work.tile([BL, 1], f32, name="ynum", tag="ynum")
            nc.vector.scalar_tensor_tensor(
                tmp64[:], e4[:], 1.0, xw[:], OP.bypass, OP.mult,
                accum_out=ynum[:])
            den = work.tile([BL, 1], f32, name="den", tag="den")
            nc.vector.reduce_sum(den[:], den4[:], axis=AX.X)
            rden = work.tile([BL, 1], f32, name="rden", tag="rden")
            nc.vector.reciprocal(rden[:], den[:])
            yt = work.tile([BL, 1], f32, name="yt", tag="yt")
            nc.vector.tensor_scalar(yt[:], ynum[:], rden[:],
                                    ytw[:, s:s + 1], OP.mult, OP.add)

            # y_tilde -> (1, 128) and K=2 matmul adds W_ih*y_tilde + bias
            ytps = ps1.tile([1, 128], f32, name="ytps", tag="ytps")
            nc.tensor.transpose(ytps[:], yt[:], ident[:])
            nc.vector.tensor_copy(ytones[0:1, :], ytps[:])
            for m in range(GB):
                nc.tensor.matmul(
                    gps[:, m * 128:(m + 1) * 128],
                    wihb[:, m * 128:(m + 1) * 128],
                    ytones[:], start=(s == 0 and m % 4 == 0),
                    stop=(m % 4 == 3))

            # ---- gate activations, order [i,f,g,o], g pre-doubled ----
            # tifg = tanh(0.5*[i,f,2g]) -> [sig-halves(i,f), tanh(g)]
            tifg = work.tile([128, 1536], f32, name="tifg", tag="tifg")
            nc.scalar.activation(tifg[:], gps[:, 0:1536], AF.Tanh, scale=0.5)
            t1 = work.tile([128, 512], f32, name="t1", tag="t1")
            nc.vector.scalar_tensor_tensor(
                t1[:], tifg[:, 512:1024], 1.0, cD[:], OP.add, OP.mult)
            t2 = work.tile([128, 512], f32, name="t2", tag="t2")
            nc.vector.scalar_tensor_tensor(
                t2[:], tifg[:, 0:512], 1.0, tifg[:, 1024:1536], OP.add,
                OP.mult)
            nc.vector.scalar_tensor_tensor(
                cD[:], t1[:], 0.5, t2[:], OP.mult, OP.add)
            nc.vector.tensor_copy(cT16[:], cD[:])
            to_t = work.tile([128, 512], f32, name="to_t", tag="to_t")
            nc.scalar.activation(to_t[:], gps[:, 1536:2048], AF.Tanh,
                                 scale=0.5)
            tcn = work.tile([128, 512], f32, name="tcn", tag="tcn")
            nc.scalar.activation(tcn[:], cD[:], AF.Tanh, scale=0.5)
            nc.vector.scalar_tensor_tensor(
                hT[:], to_t[:], 1.0, tcn[:], OP.add, OP.mult)

        # ---- final output: h.W_ffh + attn.xw2 + b_ff ----
        obps = ps1.tile([1, 128], f32, name="obps", tag="scps")
        for k in range(EC):
            nc.tensor.matmul(obps[:], wffh[:, k:k + 1],
                             hT[:, k * 128:(k + 1) * 128],
                             start=(k == 0), stop=(k == EC - 1))
        tmpf = work.tile([BL, T], f32, name="tmpf", tag="tmpf")
        a2num = work.tile([BL, 1], f32, name="a2num", tag="a2num")
        nc.vector.scalar_tensor_tensor(
            tmpf[:], e4[:], 1.0, xw2[:], OP.bypass, OP.mult,
            accum_out=a2num[:])
        a2 = work.tile([BL, 1], f32, name="a2", tag="a2")
        nc.vector.tensor_scalar(a2[:], a2num[:], rden[:], None, OP.mult)
        a2ps = ps1.tile([1, 128], f32, name="a2ps", tag="ytps")
        nc.tensor.transpose(a2ps[:], a2[:], ident[:])
        a2sb = work.tile([1, 128], f32, name="a2sb", tag="a2sb")
        nc.vector.tensor_copy(a2sb[:], a2ps[:])
        osb = work.tile([1, 128], f32, name="osb", tag="osb")
        nc.vector.scalar_tensor_tensor(
            osb[:], obps[:], float(bff), a2sb[:], OP.add, OP.add)
        nc.sync.dma_start(out_d.ap(), osb[:])

    nc.compile()
    return nc


def _prep_inputs(inputs):
    """Host-side layout prep. Returns (in_maps, scalars)."""
    f16 = np.float16
    x = np.asarray(inputs["input_encoded"], dtype=np.float32)
    yh = np.asarray(inputs["y_history"], dtype=np.float32)
    W_a1 = np.asarray(inputs["W_a1"], dtype=np.float32)
    b_a1 = np.asarray(inputs["b_a1"], dtype=np.float32)
    W_a2 = np.asarray(inputs["W_a2"], dtype=np.float32)
    b_a2 = np.asarray(inputs["b_a2"], dtype=np.float32)
    W_a3 = np.asarray(inputs["W_a3"], dtype=np.float32)
    W_ih = np.asarray(inputs["W_ih"], dtype=np.float32)
    W_hh = np.asarray(inputs["W_hh"], dtype=np.float32)
    b_ih = np.asarray(inputs["b_ih"], dtype=np.float32)
    b_hh = np.asarray(inputs["b_hh"], dtype=np.float32)
    W_fc = np.asarray(inputs["W_fc"], dtype=np.float32)
    b_fc = np.asarray(inputs["b_fc"], dtype=np.float32)
    W_ff = np.asarray(inputs["W_ff"], dtype=np.float32)

    # gate order [i, f, g, o]; g rows doubled so tanh(0.5*(2g)) = tanh(g)
    gsc = np.ones(2048, dtype=np.float32)
    gsc[1024:1536] = 2.0

    wa1t = ((W_a1.T / 2).reshape(KD, 128, 512).transpose(1, 0, 2)
            .reshape(128, KD * 512).astype(f16))
    wa2t = (W_a2.T.reshape(EC, 128, 512).transpose(1, 0, 2)
            .reshape(128, EC * 512).astype(f16))
    wa3 = W_a3[0].reshape(EC, 128).T.astype(f16).copy()
    whht = ((W_hh * gsc[:, None] / 2).T.reshape(4, 128, 2048)
            .transpose(1, 0, 2).reshape(128, 4 * 2048).astype(f16))
    wihb = np.zeros((128, 2048), dtype=np.float32)
    wihb[0] = W_ih[:, 0] * gsc
    wihb[1] = (b_ih + b_hh) * gsc
    wihb = wihb.astype(f16)
    bias1 = (b_a1 + b_a2).reshape(EC, 128).T.astype(np.float32).copy()
    wfc2 = (np.stack([W_fc[0, :512].reshape(EC, 128),
                      W_ff[0, 512:].reshape(EC, 128)], axis=-1)
            .transpose(1, 0, 2).reshape(128, 2 * EC).astype(f16))
    wffh = (W_ff[0, :512] / 2).reshape(EC, 128).T.astype(f16).copy()
    ident = np.eye(128, dtype=np.float32)
    ytinit = np.zeros((128, 128), dtype=f16)
    ytinit[1, :] = 1.0

    shared = dict(wa1t=wa1t, wa2t=wa2t, wa3=wa3, whht=whht, wihb=wihb,
                  bias1=bias1, wfc2=wfc2, wffh=wffh, ident=ident,
                  ytinit=ytinit)

    in_maps = []
    for c in range(NCORES):
        xs = x[c * BL:(c + 1) * BL]                       # (128, 64, 512)
        xt = (xs.transpose(2, 1, 0).reshape(EC, 128, T * 128)
              .transpose(1, 0, 2).reshape(128, EC * T * 128).astype(f16))
        m = dict(shared)
        m["xt"] = np.ascontiguousarray(xt)
        m["yh"] = np.ascontiguousarray(yh[c * BL:(c + 1) * BL, :, 0])
        in_maps.append(m)

    scalars = (float(W_fc[0, 512]), float(b_fc[0]), float(W_ff[0, 0]))
    return in_maps, scalars


def kernel(**inputs):
    from concourse.bass_utils import run_bass_kernel_spmd

    in_maps, _ = _prep_inputs(inputs)
    W_fc = np.asarray(inputs["W_fc"], dtype=np.float32)
    b_fc = np.asarray(inputs["b_fc"], dtype=np.float32)
    b_ff = np.asarray(inputs["b_ff"], dtype=np.float32)
    wfcy, bfc, bff = float(W_fc[0, 512]), float(b_fc[0]), float(b_ff[0])

    key = (N_STEPS, wfcy, bfc, bff)
    if key not in _PROG_CACHE:
        _PROG_CACHE[key] = _build_program(N_STEPS, wfcy, bfc, bff)
    nc = _PROG_CACHE[key]

    res = run_bass_kernel_spmd(nc, in_maps, core_ids=list(range(NCORES)))
    out = np.concatenate([res.results[c]["out"] for c in range(NCORES)],
                         axis=0).astype(np.float32)
    return out


# revision 24
# speedup vs baseline: 1.0832x; 1.0832x over previous
"""Trainium2 Bass kernel for nn_Decoder (additive-attention LSTM decoder).

Data-parallel over batch: 1024 rows split as 128 per NeuronCore across 8 cores.
All on-chip layouts keep feature dims on partitions and batch on the free dim,
so the LSTM state never needs an on-chip transpose.

v2 restructure vs baseline:
- per-tq (16-step t-quarter) score psum groups + streaming exp (no reduce_max:
  |scores| <= sum|W_a3| ~ 20 so raw exp is f32-safe) -> softmax tail collapses
- merged 8192-wide tanh per tq for tq1..3 (less ACT instr overhead); tq0 keeps
  per-chunk tanhs so the first tanh starts right after one z1 e-chunk
- z1 matmuls grouped per e-chunk (psum start/stop per m) with per-chunk psum
  evacuation, c-part emitted before h-part so it can overlap the LSTM tail
- W_hh matmuls emitted mid-phase (PE is idle between score bursts)
- LSTM gate order [i,f,g,o] with g-weights pre-doubled so one ACT op computes
  tanh(0.5*[i,f,2g]) = [sig-halves for i,f and tanh(g)] in one go
- step 0 skips z1/whh matmuls entirely (h=c=0)
"""

import os
import numpy as np

B, T, E, D = 1024, 64, 512, 512
NCORES = 8
BL = B // NCORES          # 128 batch rows per core
EC = E // 128             # 4 e-chunks
KD = (2 * D) // 128       # 8 contraction chunks for z1
GB = (4 * D) // 128       # 16 gate blocks
TQ = 4                    # t-quarters
TTQ = T // TQ             # 16 t per quarter
N_STEPS = int(os.environ.get("KERNEL_N_STEPS", str(T)))

_PROG_CACHE = {}


def _build_program(n_steps, wfcy, bfc, bff):
    from contextlib import ExitStack

    import concourse.bass as bass
    import concourse.tile as tile
    from concourse import bacc, mybir

    f16 = mybir.dt.float16
    f32 = mybir.dt.float32
    AF = mybir.ActivationFunctionType
    OP = mybir.AluOpType
    AX = mybir.AxisListType

    nc = bacc.Bacc("TRN2", target_bir_lowering=False, debug=False)

    xt_d = nc.dram_tensor("xt", (128, EC * T * 128), f16, kind="ExternalInput")
    y_d = nc.dram_tensor("yh", (BL, T), f32, kind="ExternalInput")
    wa1_d = nc.dram_tensor("wa1t", (128, KD * 512), f16, kind="ExternalInput")
    wa2_d = nc.dram_tensor("wa2t", (128, EC * 512), f16, kind="ExternalInput")
    wa3_d = nc.dram_tensor("wa3", (128, EC), f16, kind="ExternalInput")
    whh_d = nc.dram_tensor("whht", (128, 4 * 2048), f16, kind="ExternalInput")
    wihb_d = nc.dram_tensor("wihb", (128, 2048), f16, kind="ExternalInput")
    bias1_d = nc.dram_tensor("bias1", (128, EC), f32, kind="ExternalInput")
    wfc2_d = nc.dram_tensor("wfc2", (128, 2 * EC), f16, kind="ExternalInput")
    wffh_d = nc.dram_tensor("wffh", (128, EC), f16, kind="ExternalInput")
    ident_d = nc.dram_tensor("ident", (128, 128), f32, kind="ExternalInput")
    ytinit_d = nc.dram_tensor("ytinit", (128, 128), f16, kind="ExternalInput")
    out_d = nc.dram_tensor("out", (BL, 1), f32, kind="ExternalOutput")

    with tile.TileContext(nc) as tc, ExitStack() as ctx:
        const = ctx.enter_context(tc.tile_pool(name="const", bufs=1))
        z2pool = ctx.enter_context(tc.tile_pool(name="z2pool", bufs=1))

        # ---- constants into SBUF ----
        wa1t = const.tile([128, KD * 512], f16, name="wa1t", tag="wa1t")
        nc.sync.dma_start(wa1t[:], wa1_d.ap())
        whht = const.tile([128, 4 * 2048], f16, name="whht", tag="whht")
        nc.sync.dma_start(whht[:], whh_d.ap())
        wa3s = const.tile([128, EC], f16, name="wa3s", tag="wa3s")
        nc.sync.dma_start(wa3s[:], wa3_d.ap())
        wihb = const.tile([128, 2048], f16, name="wihb", tag="wihb")
        nc.sync.dma_start(wihb[:], wihb_d.ap())
        bias1 = const.tile([128, EC], f32, name="bias1", tag="bias1")
        nc.sync.dma_start(bias1[:], bias1_d.ap())
        wffh = const.tile([128, EC], f16, name="wffh", tag="wffh")
        nc.sync.dma_start(wffh[:], wffh_d.ap())
        ident = const.tile([128, 128], f32, name="ident", tag="ident")
        nc.sync.dma_start(ident[:], ident_d.ap())
        ysb = const.tile([BL, T], f32, name="ysb", tag="ysb")
        nc.sync.dma_start(ysb[:], y_d.ap())

        ytw = const.tile([BL, T], f32, name="ytw", tag="ytw")
        nc.vector.tensor_scalar(ytw[:], ysb[:], float(wfcy), float(bfc),
                                OP.mult, OP.add)

        xw = const.tile([BL, T], f32, name="xw", tag="xw")
        xw2 = const.tile([BL, T], f32, name="xw2", tag="xw2")
        zf16 = const.tile([128, 512], f16, name="zf16", tag="zf16")
        nc.vector.memset(zf16[:], 0.0)

        # z2 in transposed layout: z2all[p, c*8192 + t*128 + b]
        z2all = z2pool.tile([128, EC * T * 128], f16, name="z2all", tag="z2all")

        # ---- precompute phase: z2 = x @ W_a2.T, xw = x.W_fc, xw2 = x.W_ff2 ----
        with tc.tile_pool(name="xtp", bufs=1) as xtp, \
             tc.tile_pool(name="pcps", bufs=4, space="PSUM") as pcps:
            xts = xtp.tile([128, EC * T * 128], f16, name="xts", tag="xts")
            nc.sync.dma_start(xts[:], xt_d.ap())
            wa2t = xtp.tile([128, EC * 512], f16, name="wa2t", tag="wa2t")
            nc.sync.dma_start(wa2t[:], wa2_d.ap())
            wfc2 = xtp.tile([128, 2 * EC], f16, name="wfc2", tag="wfc2")
            nc.sync.dma_start(wfc2[:], wfc2_d.ap())

            # z2
            for cf in range(EC):
                for n in range(16):
                    zp = pcps.tile([128, 512], f32, name="zp", tag="zp")
                    for k in range(EC):
                        nc.tensor.matmul(
                            zp[:],
                            wa2t[:, k * 512 + cf * 128:k * 512 + (cf + 1) * 128],
                            xts[:, k * 8192 + n * 512:k * 8192 + (n + 1) * 512],
                            start=(k == 0), stop=(k == EC - 1))
                    nc.vector.tensor_copy(
                        z2all[:, cf * 8192 + n * 512:cf * 8192 + (n + 1) * 512],
                        zp[:])

            # xw / xw2: out[b, 2t:2t+2] = sum_e xT[e, t, b] * wfc2[e, :]
            xwp = pcps.tile([128, 2 * T], f32, name="xwp", tag="xwp", bufs=1)
            for t in range(T):
                for k in range(EC):
                    nc.tensor.matmul(
                        xwp[:, 2 * t:2 * t + 2],
                        xts[:, k * 8192 + t * 128:k * 8192 + (t + 1) * 128],
                        wfc2[:, 2 * k:2 * k + 2],
                        start=(k == 0 and t == 0),
                        stop=(k == EC - 1 and t == T - 1))
            xwp3 = xwp.rearrange("p (t two) -> p t two", two=2)
            nc.vector.tensor_copy(xw[:], xwp3[:, :, 0])
            nc.vector.tensor_copy(xw2[:], xwp3[:, :, 1])

        # Loop-phase pools open after the precompute pools released their space.
        state = ctx.enter_context(tc.tile_pool(name="state", bufs=1))
        qbig = ctx.enter_context(tc.tile_pool(name="qbig", bufs=3))
        qsm = ctx.enter_context(tc.tile_pool(name="qsm", bufs=6))
        work = ctx.enter_context(tc.tile_pool(name="work", bufs=2))
        gpsum = ctx.enter_context(
            tc.tile_pool(name="gpsum", bufs=1, space="PSUM"))
        ps1 = ctx.enter_context(tc.tile_pool(name="ps1", bufs=1, space="PSUM"))

        # ---- LSTM state (packed transposed layout, doubled h and c) ----
        hT = state.tile([128, 512], f16, name="hT", tag="hT")
        nc.vector.memset(hT[:], 0.0)
        cD = state.tile([128, 512], f32, name="cD", tag="cD")
        nc.vector.memset(cD[:], 0.0)
        cT16 = state.tile([128, 512], f16, name="cT16", tag="cT16")
        nc.vector.memset(cT16[:], 0.0)
        ytones = state.tile([128, 128], f16, name="ytones", tag="ytones")
        nc.sync.dma_start(ytones[:], ytinit_d.ap())

        e4 = None
        rden = None

        for s in range(n_steps):
            # ---- z1_T packed psum, per-m accumulation groups ----
            z1p = work.tile([128, 512], f16, name="z1p", tag="z1p")
            if s == 0:
                # h = c = 0: z1 = bias only
                nc.vector.tensor_tensor(
                    z1p.rearrange("p (m b) -> p m b", m=EC),
                    zf16.rearrange("p (m b) -> p m b", m=EC),
                    bias1.unsqueeze(2).broadcast_to((128, EC, 128)),
                    op=OP.add)
            else:
                # two independent psum half-tiles so the evac of one half
                # never blocks the other half's matmuls (coarse WAR)
                z1h = [ps1.tile([128, 256], f32, name="z1a", tag="z1a"),
                       ps1.tile([128, 256], f32, name="z1b", tag="z1b")]
                for half in range(2):
                    zt = z1h[half]
                    for mm_ in range(2):
                        m = half * 2 + mm_
                        # c-part first (cT16 ready before hT in prev tail)
                        for k in range(4, KD):
                            nc.tensor.matmul(
                                zt[:, mm_ * 128:(mm_ + 1) * 128],
                                wa1t[:, k * 512 + m * 128:k * 512 + (m + 1) * 128],
                                cT16[:, (k - 4) * 128:(k - 3) * 128],
                                start=(k == 4), stop=False)
                        for k in range(4):
                            nc.tensor.matmul(
                                zt[:, mm_ * 128:(mm_ + 1) * 128],
                                wa1t[:, k * 512 + m * 128:k * 512 + (m + 1) * 128],
                                hT[:, k * 128:(k + 1) * 128],
                                start=False, stop=(k == 3))
                    # evacuate half (2 e-chunks) with bias fold -> f16 sbuf
                    nc.vector.tensor_tensor(
                        z1p[:, half * 256:(half + 1) * 256]
                            .rearrange("p (m b) -> p m b", m=2),
                        zt.rearrange("p (m b) -> p m b", m=2),
                        bias1[:, half * 2:half * 2 + 2].unsqueeze(2)
                            .broadcast_to((128, 2, 128)),
                        op=OP.add)

            gps = gpsum.tile([128, 2048], f32, name="gps", tag="gps")
            scps = ps1.tile([128, T], f32, name="scps", tag="scps")
            e4 = work.tile([BL, T], f32, name="e4", tag="e4")
            tmp64 = work.tile([BL, T], f32, name="tmp64", tag="tmp64")

            def emit_add(qt_ap, tq, c):
                base = c * 8192 + tq * TTQ * 128
                nc.vector.tensor_tensor(
                    qt_ap.rearrange("p (t b) -> p t b", t=TTQ),
                    z2all[:, base:base + TTQ * 128]
                        .rearrange("p (t b) -> p t b", t=TTQ),
                    z1p[:, c * 128:(c + 1) * 128].unsqueeze(1)
                        .broadcast_to((128, TTQ, 128)),
                    op=OP.add)

            for tq in range(TQ):
                if tq in (0, 3):
                    # per-chunk small tiles: tq0 so the first tanh starts
                    # after one add; tq3 with per-c score rounds so the
                    # step tail is just 16 matmuls + exp
                    qs = []
                    for c in range(EC):
                        qt = qsm.tile([128, TTQ * 128], f16, name="qs",
                                      tag="qs")
                        emit_add(qt[:], tq, c)
                        nc.scalar.activation(qt[:], qt[:], AF.Tanh)
                        qs.append(qt)
                    for tt in range(TTQ):
                        t_g = tq * TTQ + tt
                        for c in range(EC):
                            nc.tensor.matmul(
                                scps[:, t_g:t_g + 1],
                                qs[c][:, tt * 128:(tt + 1) * 128],
                                wa3s[:, c:c + 1],
                                start=(c == 0), stop=(c == EC - 1))
                else:
                    qt = qbig.tile([128, TQ * TTQ * 128], f16, name="qb",
                                   tag="qb")
                    for c in range(EC):
                        emit_add(qt[:, c * 2048:(c + 1) * 2048], tq, c)
                    nc.scalar.activation(qt[:], qt[:], AF.Tanh)
                    for tt in range(TTQ):
                        t_g = tq * TTQ + tt
                        for c in range(EC):
                            nc.tensor.matmul(
                                scps[:, t_g:t_g + 1],
                                qt[:, c * 2048 + tt * 128:c * 2048 + (tt + 1) * 128],
                                wa3s[:, c:c + 1],
                                start=(c == 0), stop=(c == EC - 1))

                if tq == 0 and s > 0:
                    # W_hh gate matmuls: PE is idle between score bursts
                    for m in range(GB):
                        for k in range(4):
                            nc.tensor.matmul(
                                gps[:, m * 128:(m + 1) * 128],
                                whht[:, k * 2048 + m * 128:k * 2048 + (m + 1) * 128],
                                hT[:, k * 128:(k + 1) * 128],
                                start=(k == 0 and m % 4 == 0), stop=False)

            # ---- softmax pieces: one exp over all 64 scores ----
            # (no max subtraction needed: |s| <= sum|W_a3| ~ 20, f32-safe)
            den = work.tile([BL, 1], f32, name="den", tag="den")
            nc.scalar.activation(e4[:], scps[:], AF.Exp, accum_out=den[:])
            ynum = work.tile([BL, 1], f32, name="ynum", tag="ynum")
            nc.vector.scalar_tensor_tensor(
                tmp64[:], e4[:], 1.0, xw[:], OP.bypass, OP.mult,
                accum_out=ynum[:])
            rden = work.tile([BL, 1], f32, name="rden", tag="rden")
            nc.vector.reciprocal(rden[:], den[:])
            yt = work.tile([BL, 1], f32, name="yt", tag="yt")
            nc.vector.tensor_scalar(yt[:], ynum[:], rden[:],
                                    ytw[:, s:s + 1], OP.mult, OP.add)

            # y_tilde -> (1, 128) and K=2 matmul adds W_ih*y_tilde + bias
            ytps = ps1.tile([1, 128], f32, name="ytps", tag="ytps")
            nc.tensor.transpose(ytps[:], yt[:], ident[:])
            nc.vector.tensor_copy(ytones[0:1, :], ytps[:])
            for m in range(GB):
                nc.tensor.matmul(
                    gps[:, m * 128:(m + 1) * 128],
                    wihb[:, m * 128:(m + 1) * 128],
                    ytones[:], start=(s == 0 and m % 4 == 0),
                    stop=(m % 4 == 3))

            # ---- gate activations, order [f,i,g,o], g pre-doubled ----
            # split into [f] / [i,g] / [o] pieces so the c-update chain
            # starts as soon as the f-part of the gates psum is ready
            tf = work.tile([128, 512], f32, name="tf", tag="tf")
            nc.scalar.activation(tf[:], gps[:, 0:512], AF.Tanh, scale=0.5)
            t1 = work.tile([128, 512], f32, name="t1", tag="t1")
            nc.vector.scalar_tensor_tensor(
                t1[:], tf[:], 1.0, cD[:], OP.add, OP.mult)
            tig = work.tile([128, 1024], f32, name="tig", tag="tig")
            nc.scalar.activation(tig[:], gps[:, 512:1536], AF.Tanh, scale=0.5)
            t2 = work.tile([128, 512], f32, name="t2", tag="t2")
            nc.vector.scalar_tensor_tensor(
                t2[:], tig[:, 0:512], 1.0, tig[:, 512:1024], OP.add,
                OP.mult)
            nc.vector.scalar_tensor_tensor(
                cD[:], t1[:], 0.5, t2[:], OP.mult, OP.add)
            nc.vector.tensor_copy(cT16[:], cD[:])
            to_t = work.tile([128, 512], f32, name="to_t", tag="to_t")
            nc.scalar.activation(to_t[:], gps[:, 1536:2048], AF.Tanh,
                                 scale=0.5)
            tcn = work.tile([128, 512], f32, name="tcn", tag="tcn")
            nc.scalar.activation(tcn[:], cD[:], AF.Tanh, scale=0.5)
            nc.vector.scalar_tensor_tensor(
                hT[:], to_t[:], 1.0, tcn[:], OP.add, OP.mult)

        # ---- final output: h.W_ffh + attn.xw2 + b_ff ----
        obps = ps1.tile([1, 128], f32, name="obps", tag="scps")
        for k in range(EC):
            nc.tensor.matmul(obps[:], wffh[:, k:k + 1],
                             hT[:, k * 128:(k + 1) * 128],
                             start=(k == 0), stop=(k == EC - 1))
        tmpf = work.tile([BL, T], f32, name="tmpf", tag="tmpf")
        a2num = work.tile([BL, 1], f32, name="a2num", tag="a2num")
        nc.vector.scalar_tensor_tensor(
            tmpf[:], e4[:], 1.0, xw2[:], OP.bypass, OP.mult,
            accum_out=a2num[:])
        a2 = work.tile([BL, 1], f32, name="a2", tag="a2")
        nc.vector.tensor_scalar(a2[:], a2num[:], rden[:], None, OP.mult)
        a2ps = ps1.tile([1, 128], f32, name="a2ps", tag="ytps")
        nc.tensor.transpose(a2ps[:], a2[:], ident[:])
        a2sb = work.tile([1, 128], f32, name="a2sb", tag="a2sb")
        nc.vector.tensor_copy(a2sb[:], a2ps[:])
        osb = work.tile([1, 128], f32, name="osb", tag="osb")
        nc.vector.scalar_tensor_tensor(
            osb[:], obps[:], float(bff), a2sb[:], OP.add, OP.add)
        nc.sync.dma_start(out_d.ap(), osb[:])

    nc.compile()
    return nc


def _prep_inputs(inputs):
    """Host-side layout prep. Returns (in_maps, scalars)."""
    f16 = np.float16
    x = np.asarray(inputs["input_encoded"], dtype=np.float32)
    yh = np.asarray(inputs["y_history"], dtype=np.float32)
    W_a1 = np.asarray(inputs["W_a1"], dtype=np.float32)
    b_a1 = np.asarray(inputs["b_a1"], dtype=np.float32)
    W_a2 = np.asarray(inputs["W_a2"], dtype=np.float32)
    b_a2 = np.asarray(inputs["b_a2"], dtype=np.float32)
    W_a3 = np.asarray(inputs["W_a3"], dtype=np.float32)
    W_ih = np.asarray(inputs["W_ih"], dtype=np.float32)
    W_hh = np.asarray(inputs["W_hh"], dtype=np.float32)
    b_ih = np.asarray(inputs["b_ih"], dtype=np.float32)
    b_hh = np.asarray(inputs["b_hh"], dtype=np.float32)
    W_fc = np.asarray(inputs["W_fc"], dtype=np.float32)
    b_fc = np.asarray(inputs["b_fc"], dtype=np.float32)
    W_ff = np.asarray(inputs["W_ff"], dtype=np.float32)

    # gate order [f, i, g, o]; g rows doubled so tanh(0.5*(2g)) = tanh(g)
    order = np.r_[512:1024, 0:512, 1024:1536, 1536:2048]
    gsc = np.ones(2048, dtype=np.float32)
    gsc[1024:1536] = 2.0

    wa1t = ((W_a1.T / 2).reshape(KD, 128, 512).transpose(1, 0, 2)
            .reshape(128, KD * 512).astype(f16))
    wa2t = (W_a2.T.reshape(EC, 128, 512).transpose(1, 0, 2)
            .reshape(128, EC * 512).astype(f16))
    wa3 = W_a3[0].reshape(EC, 128).T.astype(f16).copy()
    whht = ((W_hh[order] * gsc[:, None] / 2).T.reshape(4, 128, 2048)
            .transpose(1, 0, 2).reshape(128, 4 * 2048).astype(f16))
    wihb = np.zeros((128, 2048), dtype=np.float32)
    wihb[0] = W_ih[order, 0] * gsc
    wihb[1] = (b_ih + b_hh)[order] * gsc
    wihb = wihb.astype(f16)
    bias1 = (b_a1 + b_a2).reshape(EC, 128).T.astype(np.float32).copy()
    wfc2 = (np.stack([W_fc[0, :512].reshape(EC, 128),
                      W_ff[0, 512:].reshape(EC, 128)], axis=-1)
            .transpose(1, 0, 2).reshape(128, 2 * EC).astype(f16))
    wffh = (W_ff[0, :512] / 2).reshape(EC, 128).T.astype(f16).copy()
    ident = np.eye(128, dtype=np.float32)
    ytinit = np.zeros((128, 128), dtype=f16)
    ytinit[1, :] = 1.0

    shared = dict(wa1t=wa1t, wa2t=wa2t, wa3=wa3, whht=whht, wihb=wihb,
                  bias1=bias1, wfc2=wfc2, wffh=wffh, ident=ident,
                  ytinit=ytinit)

    in_maps = []
    for c in range(NCORES):
        xs = x[c * BL:(c + 1) * BL]                       # (128, 64, 512)
        xt = (xs.transpose(2, 1, 0).reshape(EC, 128, T * 128)
              .transpose(1, 0, 2).reshape(128, EC * T * 128).astype(f16))
        m = dict(shared)
        m["xt"] = np.ascontiguousarray(xt)
        m["yh"] = np.ascontiguousarray(yh[c * BL:(c + 1) * BL, :, 0])
        in_maps.append(m)

    scalars = (float(W_fc[0, 512]), float(b_fc[0]), float(W_ff[0, 0]))
    return in_maps, scalars


def kernel(**inputs):
    from concourse.bass_utils import run_bass_kernel_spmd

    in_maps, _ = _prep_inputs(inputs)
    W_fc = np.asarray(inputs["W_fc"], dtype=np.float32)
    b_fc = np.asarray(inputs["b_fc"], dtype=np.float32)
    b_ff = np.asarray(inputs["b_ff"], dtype=np.float32)
    wfcy, bfc, bff = float(W_fc[0, 512]), float(b_fc[0]), float(b_ff[0])

    key = (N_STEPS, wfcy, bfc, bff)
    if key not in _PROG_CACHE:
        _PROG_CACHE[key] = _build_program(N_STEPS, wfcy, bfc, bff)
    nc = _PROG_CACHE[key]

    res = run_bass_kernel_spmd(nc, in_maps, core_ids=list(range(NCORES)))
    out = np.concatenate([res.results[c]["out"] for c in range(NCORES)],
                         axis=0).astype(np.float32)
    return out
